# revision 23
# baseline (speedup 1.0000x reference)
"""Trainium2 Bass kernel for nn_BinaryAttentionB (binary-quantised attention).

Math notes (vs. the jax reference):
  - qq . kk with qq=[qw1,qw2,qw1,qw2], kk=[kw1,kw1,kw2,kw2] collapses to
    (qw1+qw2).(kw1+kw2): a single 64-dim contraction with
    qs = (2*b1-1)*w1 + (2*b2-1)*w2 = 2*(b1*w1 + b2*w2) - 1  (w1+w2 == 1).
  - |scores| <= 64/8 = 8, so softmax == exp(s)/sum(exp(s)) is fp32-safe
    without the max subtraction.  This lets us compute S^T tiles on the PE
    (k on partitions, q on free), exp them on ACT straight out of PSUM, and
    feed P^T directly to the PV matmul.  A ones-column appended to V makes
    the PV matmul also produce the softmax denominator.

Sharding: 8 cores, data-parallel over the B*H=24 head-batch axis: core c
handles batch b=c//2, heads [g*3,(g+1)*3) with g=c%2.  Weights are sliced
per-core and pre-transposed on host; x[b] is pre-transposed on host.
"""

import sys
import types

import numpy as np

# ---------------------------------------------------------------------------
# Environment workarounds (self-contained on purpose)
# ---------------------------------------------------------------------------


def _patch_tile_tail_drain():
    """walrus in this image rejects >1 sem-wait per instruction; Tile's tail
    drain aggregates one wait per outstanding proc.  Split them across
    consecutive SP drains."""
    import concourse.tile as tile_mod
    from concourse import mybir
    from concourse.vector_clock import ScopedClock

    if getattr(tile_mod.TileContext, "_drain_split_patched", False):
        return

    def _drain_and_barrier(self, tick_clock, wait_clock):
        drain_inst = self.nc.sync.drain()
        wait_clock.add_sem_waits(
            drain_inst.ins, ScopedClock({None: tick_clock.global_clock})
        )
        si = drain_inst.ins.sync_info
        waits = list(si.on_wait or []) if si is not None else []
        if len(waits) > 1:
            si.on_wait = waits[:1]
            for w in waits[1:]:
                d2 = self.nc.sync.drain()
                if d2.ins.sync_info is None:
                    d2.ins.sync_info = mybir.SyncInfo(on_wait=[w], on_update=[])
                else:
                    d2.ins.sync_info.on_wait = [w]
        self.nc.all_engine_barrier()
        assert self.sems is not None
        popped = self.nc._tile_sem_poison_stack.pop()
        assert popped is self._sem_poison
        self.nc.clear_and_free_semaphores(list(self.sems.allocated().values()))
        self.nc.all_engine_barrier()

    tile_mod.TileContext._drain_and_barrier = _drain_and_barrier
    tile_mod.TileContext._drain_split_patched = True


def _split_multiwaits(nc):
    """walrus here allows only one sem-wait per instruction: move extra waits
    onto same-engine NoOps inserted just before the offending instruction."""
    from concourse import mybir

    n = 0
    for f in nc.m.functions:
        for blk in f.blocks:
            il = blk.instructions
            i = 0
            while i < len(il):
                inst = il[i]
                si = inst.sync_info
                if si is not None and si.on_wait and len(si.on_wait) > 1:
                    waits = list(si.on_wait)
                    si.on_wait = waits[-1:]
                    for w in waits[:-1]:
                        nop = mybir.InstNoOp(
                            name=f"mwsplit-{n}",
                            engine=inst.engine,
                            sync_info=mybir.SyncInfo(on_wait=[w], on_update=[]),
                            bass_nofuse=True,
                        )
                        n += 1
                        il.insert(i, nop)
                        i += 1
                i += 1
    return n


def _install_ntff_hook():
    """Optional: register the NTFF profile hook so trace=True works (the
    image's antenv lacks axon_hooks; rebuild it from the boot helper)."""
    if "antenv.axon_hooks" in sys.modules:
        return
    try:
        from trn_agent_boot.trn_boot import _ntff_profile_via_ctypes

        hook = _ntff_profile_via_ctypes("/opt/axon/libaxon_pjrt.so")
        mod = types.ModuleType("antenv.axon_hooks")
        mod.get_axon_ntff_profile_hook = lambda: hook
        mod.set_axon_ntff_profile_hook = lambda h: None
        sys.modules["antenv.axon_hooks"] = mod
    except Exception:
        pass


# ---------------------------------------------------------------------------
# Problem constants (hardcoded per the harness contract)
# ---------------------------------------------------------------------------
B, S, D = 4, 2048, 384
H, DH, DV = 6, 64, 16
NCORES = 8
NH = 3          # heads per core
GO = H // 2 * DH // 64 * 0 + 192  # per-core q/k out width = NH*DH = 192
VO = NH * DV    # 48
P = 128
ST = S // P     # 16 s-tiles
KT = D // P     # 3 contraction tiles for the projections
QC = 4          # q chunks of 512
QW = 512
SCALE = 1.0 / 8.0  # 1/sqrt(DH)


def _build_nc():
    import concourse.bass as bass
    import concourse.tile as tile
    from concourse import mybir
    from concourse.masks import make_identity

    f32 = mybir.dt.float32
    f32r = mybir.dt.float32r
    Alu = mybir.AluOpType
    Act = mybir.ActivationFunctionType

    nc = bass.Bass("TRN2", target_bir_lowering=False, debug=False)

    WO = 2 * GO + VO  # 432: q|k|v projection columns fused
    xT = nc.dram_tensor("xT", [D, S], f32, kind="ExternalInput").ap()
    wT = nc.dram_tensor("wT", [D, WO], f32, kind="ExternalInput").ap()
    bias = nc.dram_tensor("bias", [1, WO], f32, kind="ExternalInput").ap()
    u_d = {}
    for j in range(NH):
        for nm in ("uq1", "uq2", "uk1", "uk2"):
            u_d[(nm, j)] = nc.dram_tensor(
                f"{nm}_{j}", [S, DH], f32, kind="ExternalInput"
            ).ap()
    out_d = nc.dram_tensor("out", [S, VO], f32, kind="ExternalOutput").ap()

    with tile.TileContext(nc) as tc:
        with (
            tc.tile_pool(name="const", bufs=1) as const_pool,
            tc.tile_pool(name="persist", bufs=1) as persist,
            tc.tile_pool(name="work", bufs=2) as work,
            tc.tile_pool(name="small", bufs=4) as small,
        ):
            identity = const_pool.tile([P, P], f32)
            make_identity(nc, identity)
            onesc = const_pool.tile([P, 1], f32)
            nc.vector.memset(onesc, 1.0)
            ones1 = const_pool.tile([1, P], f32)
            nc.vector.memset(ones1, 1.0)

            w_sb = persist.tile([P, KT, WO], f32)
            nc.sync.dma_start(out=w_sb, in_=wT.rearrange("(k p) o -> p k o", p=P))
            b_sb = persist.tile([1, WO], f32)
            nc.sync.dma_start(out=b_sb, in_=bias)
            xT_sb = persist.tile([P, KT, S], f32)
            xv = xT.rearrange("(k p) s -> p k s", p=P)
            for g4 in range(4):
                ssl = slice(g4 * (S // 4), (g4 + 1) * (S // 4))
                nc.sync.dma_start(out=xT_sb[:, :, ssl], in_=xv[:, :, ssl])

            p_q = persist.tile([P, ST, GO], f32)
            p_k = persist.tile([P, ST, GO], f32)
            VW = 32  # PV stationary padded to a 32-col group
            zeroc = const_pool.tile([P, 1], f32)
            nc.vector.memset(zeroc, 0.0)
            v_all = persist.tile([P, ST, NH, VW], f32r)
            nc.vector.tensor_copy(
                v_all[:, :, :, DV + 1 : VW],
                zeroc[:, None, None, :].to_broadcast([P, ST, NH, VW - DV - 1]),
            )
            nc.vector.tensor_copy(
                v_all[:, :, :, DV : DV + 1],
                onesc[:, None, None, :].to_broadcast([P, ST, NH, 1]),
            )

            # ---------------- phase 1: fused q|k|v projection ---------------
            with tc.tile_pool(name="pjp", bufs=3, space="PSUM") as pjp:
                for st in range(ST):
                    xs = xT_sb[:, :, st * P : (st + 1) * P]
                    pp = pjp.tile([P, WO], f32, name=f"pp{st}", tag="pj")
                    for ki in range(KT):
                        nc.tensor.matmul(
                            pp,
                            lhsT=xs[:, ki, :],
                            rhs=w_sb[:, ki, :],
                            start=(ki == 0),
                            stop=False,
                        )
                    nc.tensor.matmul(pp, lhsT=ones1, rhs=b_sb, start=False, stop=True)
                    nc.scalar.activation(p_q[:, st, :], pp[:, 0:GO], Act.Tanh)
                    nc.scalar.activation(p_k[:, st, :], pp[:, GO : 2 * GO], Act.Tanh)
                    nc.scalar.activation(
                        p_q[:, st, :], p_q[:, st, :], Act.Copy, bias=0.5, scale=0.5
                    )
                    nc.scalar.activation(
                        p_k[:, st, :], p_k[:, st, :], Act.Copy, bias=0.5, scale=0.5
                    )
                    nc.vector.tensor_copy(
                        out=v_all[:, st, :, 0:DV],
                        in_=pp[:, 2 * GO : WO].rearrange("p (h v) -> p h v", h=NH),
                    )

            # ---------------- phase 2: per-head quantise + attention --------
            with (
                tc.tile_pool(name="trp", bufs=2, space="PSUM") as trp,
                tc.tile_pool(name="ssp", bufs=2, space="PSUM") as ssp,
                tc.tile_pool(name="osp", bufs=2, space="PSUM") as osp,
            ):
                for j in range(NH):
                    qskT = {}
                    for side in ("q", "k"):
                        p_all = p_q if side == "q" else p_k
                        p_h = p_all[:, :, j * DH : (j + 1) * DH]
                        u1 = work.tile([P, ST, DH], f32, name=f"u1{side}{j}", tag=f"u1{side}")
                        nc.sync.dma_start(
                            out=u1,
                            in_=u_d[(f"u{side}1", j)].rearrange("(t p) d -> p t d", p=P),
                        )
                        u2 = work.tile([P, ST, DH], f32, name=f"u2{side}{j}", tag=f"u2{side}")
                        nc.sync.dma_start(
                            out=u2,
                            in_=u_d[(f"u{side}2", j)].rearrange("(t p) d -> p t d", p=P),
                        )
                        b1 = work.tile([P, ST, DH], f32, name=f"b1{side}{j}", tag=f"b1{side}")
                        b2 = work.tile([P, ST, DH], f32, name=f"b2{side}{j}", tag=f"b2{side}")
                        if side == "q":
                            qs2 = work.tile([P, ST, 2, DH], f32, name=f"qs2{j}", tag="qs2")
                        # quantise in groups of 4 s-tiles so the first
                        # transposes/matmuls can start while later groups are
                        # still on DVE (kills the serial head-0 bubble)
                        NG, GW = 4, ST // 4
                        for g in range(NG):
                            sl = slice(g * GW, (g + 1) * GW)
                            p_g = p_h[:, sl, :]
                            b1g, b2g, u1g, u2g = b1[:, sl, :], b2[:, sl, :], u1[:, sl, :], u2[:, sl, :]
                            nc.vector.tensor_tensor(b1g, u1g, p_g, Alu.is_lt)
                            nc.vector.tensor_tensor(b2g, u2g, p_g, Alu.is_lt)
                            nc.vector.tensor_tensor(u1g, p_g, b1g, Alu.subtract)
                            d1 = small.tile([P, GW], f32, name=f"d1{side}{j}{g}", tag=f"d1{side}")
                            nc.vector.tensor_reduce(
                                d1, u1g, op=Alu.add, axis=mybir.AxisListType.X,
                                apply_absolute_value=True,
                            )
                            nc.vector.tensor_tensor(u2g, p_g, b2g, Alu.subtract)
                            d2 = small.tile([P, GW], f32, name=f"d2{side}{j}{g}", tag=f"d2{side}")
                            nc.vector.tensor_reduce(
                                d2, u2g, op=Alu.add, axis=mybir.AxisListType.X,
                                apply_absolute_value=True,
                            )
                            # d = mean + 1e-12 ; w1 = d2/(d1+d2), w2 = d1/(d1+d2)
                            nc.vector.tensor_scalar(
                                out=d1, in0=d1, scalar1=1.0 / DH, scalar2=1e-12,
                                op0=Alu.mult, op1=Alu.add,
                            )
                            nc.vector.tensor_scalar(
                                out=d2, in0=d2, scalar1=1.0 / DH, scalar2=1e-12,
                                op0=Alu.mult, op1=Alu.add,
                            )
                            ds = small.tile([P, GW], f32, name=f"ds{side}{j}{g}", tag=f"ds{side}")
                            nc.vector.tensor_tensor(ds, d1, d2, Alu.add)
                            nc.vector.reciprocal(ds, ds)
                            w1 = small.tile([P, GW], f32, name=f"w1{side}{j}{g}", tag=f"w1{side}")
                            nc.vector.tensor_tensor(w1, d2, ds, Alu.mult)
                            w2 = small.tile([P, GW], f32, name=f"w2{side}{j}{g}", tag=f"w2{side}")
                            nc.vector.tensor_tensor(w2, d1, ds, Alu.mult)
                            # qs = 2*(b1*w1 + b2*w2) - 1
                            nc.vector.tensor_tensor(
                                b1g, b1g, w1[:, :, None].to_broadcast([P, GW, DH]), Alu.mult
                            )
                            nc.vector.tensor_tensor(
                                b2g, b2g, w2[:, :, None].to_broadcast([P, GW, DH]), Alu.mult
                            )
                            if side == "q":
                                q2g = qs2[:, sl, :, :]
                                nc.vector.tensor_tensor(q2g[:, :, 0, :], b1g, b2g, Alu.add)
                                nc.vector.tensor_scalar(
                                    out=q2g[:, :, 0, :], in0=q2g[:, :, 0, :],
                                    scalar1=2.0, scalar2=-1.0,
                                    op0=Alu.mult, op1=Alu.add,
                                )
                                nc.vector.tensor_copy(q2g[:, :, 1, :], q2g[:, :, 0, :])
                            else:
                                nc.vector.tensor_tensor(b1g, b1g, b2g, Alu.add)
                                nc.vector.tensor_scalar(
                                    out=b1g, in0=b1g, scalar1=2.0, scalar2=-1.0,
                                    op0=Alu.mult, op1=Alu.add,
                                )
                        qs = b1
                        # Transpose [s, dh] -> [dh, s] via PE on [128,128]
                        # blocks.  For q: transpose a free-dim-duplicated tile
                        # so qsT lands identically on partitions 0-63 and
                        # 64-127 (feeds both concurrent row-group matmuls).
                        # For k: transpose s-tile PAIRS so even k-tiles land on
                        # partitions 0-63 and odd ones on 64-127 (packed ksT).
                        if side == "q":
                            qsT = work.tile([P, S], f32r, name=f"qsT{j}", tag="qsT")
                            for st in range(ST):
                                tr = trp.tile([P, P], f32, name=f"trq{j}{st}", tag="tr")
                                nc.tensor.transpose(tr, qs2[:, st, :, :], identity)
                                nc.vector.tensor_copy(
                                    out=qsT[:, st * P : (st + 1) * P], in_=tr
                                )
                            qskT[side] = qsT
                        else:
                            ksT = work.tile([P, ST // 2, P], f32r, name=f"ksT{j}", tag="ksT")
                            for kp in range(ST // 2):
                                tr = trp.tile([P, P], f32, name=f"trk{j}{kp}", tag="tr")
                                nc.tensor.transpose(
                                    tr, qs[:, 2 * kp : 2 * kp + 2, :], identity
                                )
                                nc.vector.tensor_copy(out=ksT[:, kp, :], in_=tr)
                            qskT[side] = ksT

                    qsT, ksT = qskT["q"], qskT["k"]
                    # attention for head j: scores row-packed (two k-tiles in
                    # row groups 0/64 concurrently), PV accumulates [32, QW]
                    for qc in range(QC):
                        rhs_q = qsT[:, qc * QW : (qc + 1) * QW]
                        o_psA = osp.tile([32, QW], f32, name=f"oa{j}{qc}", tag="oa", bufs=1)
                        o_psB = osp.tile([32, QW], f32, name=f"ob{j}{qc}", tag="obk", bufs=1)
                        for kp in range(ST // 2):
                            s_ps = ssp.tile([P, 2, QW], f32, name=f"s{j}{qc}{kp}", tag="s")
                            for h2 in range(2):
                                base = h2 * DH
                                nc.tensor.matmul(
                                    s_ps[:, h2, :],
                                    lhsT=ksT[base : base + DH, kp, :],
                                    rhs=rhs_q[base : base + DH, :],
                                    start=True,
                                    stop=True,
                                )
                            p_sb = work.tile([P, 2, QW], f32r, name=f"p{j}{qc}{kp}", tag="p")
                            nc.scalar.activation(p_sb, s_ps, Act.Exp, scale=SCALE)
                            for h2 in range(2):
                                kt = kp * 2 + h2
                                nc.tensor.matmul(
                                    o_psA,
                                    lhsT=v_all[0:DH, kt, j, :],
                                    rhs=p_sb[0:DH, h2, :],
                                    start=(kt == 0),
                                    stop=(kt == ST - 1),
                                )
                                nc.tensor.matmul(
                                    o_psB,
                                    lhsT=v_all[DH:P, kt, j, :],
                                    rhs=p_sb[DH:P, h2, :],
                                    start=(kt == 0),
                                    stop=(kt == ST - 1),
                                )
                        oT = work.tile([DV + 1, QW], f32, name=f"oT{j}{qc}", tag="oT")
                        nc.vector.tensor_copy(oT, o_psA[0 : DV + 1, :])
                        nc.vector.tensor_tensor(
                            oT, oT, o_psB[0 : DV + 1, :], Alu.add
                        )
                        for blk in range(QW // P):
                            tro = trp.tile([P, DV + 1], f32, name=f"tro{j}{qc}{blk}", tag="tr")
                            nc.tensor.transpose(
                                tro,
                                oT[:, blk * P : (blk + 1) * P],
                                identity[: DV + 1, : DV + 1],
                            )
                            rec = small.tile([P, 1], f32, name=f"rec{j}{qc}{blk}", tag="rec")
                            nc.vector.reciprocal(rec, tro[:, DV : DV + 1])
                            o_sb = small.tile([P, DV], f32, name=f"ob{j}{qc}{blk}", tag="ob")
                            nc.vector.tensor_scalar_mul(o_sb, tro[:, 0:DV], rec)
                            s0 = qc * QW + blk * P
                            nc.sync.dma_start(
                                out=out_d[s0 : s0 + P, j * DV : (j + 1) * DV],
                                in_=o_sb,
                            )
    _split_multiwaits(nc)
    return nc


_NC = None


def _get_nc():
    global _NC
    if _NC is None:
        _patch_tile_tail_drain()
        _NC = _build_nc()
    return _NC


def _shard_inputs(inputs):
    x = np.asarray(inputs["x"], dtype=np.float32)
    Wq = np.asarray(inputs["Wq"], dtype=np.float32)
    bq = np.asarray(inputs["bq"], dtype=np.float32)
    Wk = np.asarray(inputs["Wk"], dtype=np.float32)
    bk = np.asarray(inputs["bk"], dtype=np.float32)
    Wv = np.asarray(inputs["Wv"], dtype=np.float32)
    bv = np.asarray(inputs["bv"], dtype=np.float32)
    us = {nm: np.asarray(inputs[nm], dtype=np.float32)
          for nm in ("u_q1", "u_q2", "u_k1", "u_k2")}

    in_maps = []
    for c in range(NCORES):
        b, g = divmod(c, 2)
        wT = np.concatenate(
            [
                Wq[g * GO : (g + 1) * GO, :].T,
                Wk[g * GO : (g + 1) * GO, :].T,
                Wv[g * VO : (g + 1) * VO, :].T,
            ],
            axis=1,
        )
        bias = np.concatenate(
            [
                bq[g * GO : (g + 1) * GO],
                bk[g * GO : (g + 1) * GO],
                bv[g * VO : (g + 1) * VO],
            ]
        ).reshape(1, -1)
        m = {
            "xT": np.ascontiguousarray(x[b].T),
            "wT": np.ascontiguousarray(wT),
            "bias": np.ascontiguousarray(bias),
        }
        for j in range(NH):
            bh = b * H + g * NH + j
            m[f"uq1_{j}"] = np.ascontiguousarray(us["u_q1"][bh])
            m[f"uq2_{j}"] = np.ascontiguousarray(us["u_q2"][bh])
            m[f"uk1_{j}"] = np.ascontiguousarray(us["u_k1"][bh])
            m[f"uk2_{j}"] = np.ascontiguousarray(us["u_k2"][bh])
        in_maps.append(m)
    return in_maps


def _run(inputs, trace=False, tmpdir=None):
    from concourse.bass_utils import run_bass_kernel_spmd

    if trace:
        _install_ntff_hook()
    nc = _get_nc()
    in_maps = _shard_inputs(inputs)
    kw = {}
    if trace:
        kw["trace"] = True
        if tmpdir is not None:
            kw["tmpdir"] = tmpdir
    res = run_bass_kernel_spmd(nc, in_maps, core_ids=list(range(NCORES)), **kw)
    out = np.zeros((B, S, H * DV), dtype=np.float32)
    for c in range(NCORES):
        b, g = divmod(c, 2)
        out[b, :, g * VO : (g + 1) * VO] = res.results[c]["out"]
    return (out,), res


def kernel(**inputs):
    out, _ = _run(inputs, trace=False)
    return out


def kernel_profiled(tmpdir=None, **inputs):
    out, res = _run(inputs, trace=True, tmpdir=tmpdir)
    return out, res.exec_time_ns


# revision 24
# speedup vs baseline: 1.1395x; 1.1395x over previous
"""Trainium2 Bass kernel for nn_BinaryAttentionB (binary-quantised attention).

Math notes (vs. the jax reference):
  - qq . kk with qq=[qw1,qw2,qw1,qw2], kk=[kw1,kw1,kw2,kw2] collapses to
    (qw1+qw2).(kw1+kw2): a single 64-dim contraction with
    qs = (2*b1-1)*w1 + (2*b2-1)*w2 = 2*(b1*w1 + b2*w2) - 1  (w1+w2 == 1).
  - |scores| <= 64/8 = 8, so softmax == exp(s)/sum(exp(s)) is fp32-safe
    without the max subtraction.  This lets us compute S^T tiles on the PE
    (k on partitions, q on free), exp them on ACT straight out of PSUM, and
    feed P^T directly to the PV matmul.  A ones-column appended to V makes
    the PV matmul also produce the softmax denominator.

Sharding: 8 cores, data-parallel over the B*H=24 head-batch axis: core c
handles batch b=c//2, heads [g*3,(g+1)*3) with g=c%2.  Weights are sliced
per-core and pre-transposed on host; x[b] is pre-transposed on host.
"""

import sys
import types

import numpy as np

# ---------------------------------------------------------------------------
# Environment workarounds (self-contained on purpose)
# ---------------------------------------------------------------------------


def _patch_tile_tail_drain():
    """walrus in this image rejects >1 sem-wait per instruction; Tile's tail
    drain aggregates one wait per outstanding proc.  Split them across
    consecutive SP drains."""
    import concourse.tile as tile_mod
    from concourse import mybir
    from concourse.vector_clock import ScopedClock

    if getattr(tile_mod.TileContext, "_drain_split_patched", False):
        return

    def _drain_and_barrier(self, tick_clock, wait_clock):
        drain_inst = self.nc.sync.drain()
        wait_clock.add_sem_waits(
            drain_inst.ins, ScopedClock({None: tick_clock.global_clock})
        )
        si = drain_inst.ins.sync_info
        waits = list(si.on_wait or []) if si is not None else []
        if len(waits) > 1:
            si.on_wait = waits[:1]
            for w in waits[1:]:
                d2 = self.nc.sync.drain()
                if d2.ins.sync_info is None:
                    d2.ins.sync_info = mybir.SyncInfo(on_wait=[w], on_update=[])
                else:
                    d2.ins.sync_info.on_wait = [w]
        self.nc.all_engine_barrier()
        assert self.sems is not None
        popped = self.nc._tile_sem_poison_stack.pop()
        assert popped is self._sem_poison
        self.nc.clear_and_free_semaphores(list(self.sems.allocated().values()))
        self.nc.all_engine_barrier()

    tile_mod.TileContext._drain_and_barrier = _drain_and_barrier
    tile_mod.TileContext._drain_split_patched = True


def _split_multiwaits(nc):
    """walrus here allows only one sem-wait per instruction: move extra waits
    onto same-engine NoOps inserted just before the offending instruction."""
    from concourse import mybir

    n = 0
    for f in nc.m.functions:
        for blk in f.blocks:
            il = blk.instructions
            i = 0
            while i < len(il):
                inst = il[i]
                si = inst.sync_info
                if si is not None and si.on_wait and len(si.on_wait) > 1:
                    waits = list(si.on_wait)
                    si.on_wait = waits[-1:]
                    for w in waits[:-1]:
                        nop = mybir.InstNoOp(
                            name=f"mwsplit-{n}",
                            engine=inst.engine,
                            sync_info=mybir.SyncInfo(on_wait=[w], on_update=[]),
                            bass_nofuse=True,
                        )
                        n += 1
                        il.insert(i, nop)
                        i += 1
                i += 1
    return n


def _install_ntff_hook():
    """Optional: register the NTFF profile hook so trace=True works (the
    image's antenv lacks axon_hooks; rebuild it from the boot helper)."""
    if "antenv.axon_hooks" in sys.modules:
        return
    try:
        from trn_agent_boot.trn_boot import _ntff_profile_via_ctypes

        hook = _ntff_profile_via_ctypes("/opt/axon/libaxon_pjrt.so")
        mod = types.ModuleType("antenv.axon_hooks")
        mod.get_axon_ntff_profile_hook = lambda: hook
        mod.set_axon_ntff_profile_hook = lambda h: None
        sys.modules["antenv.axon_hooks"] = mod
    except Exception:
        pass


# ---------------------------------------------------------------------------
# Problem constants (hardcoded per the harness contract)
# ---------------------------------------------------------------------------
B, S, D = 4, 2048, 384
H, DH, DV = 6, 64, 16
NCORES = 8
NH = 3          # heads per core
GO = H // 2 * DH // 64 * 0 + 192  # per-core q/k out width = NH*DH = 192
VO = NH * DV    # 48
P = 128
ST = S // P     # 16 s-tiles
KT = D // P     # 3 contraction tiles for the projections
QC = 4          # q chunks of 512
QW = 512
SCALE = 1.0 / 8.0  # 1/sqrt(DH)


def _build_nc():
    import concourse.bass as bass
    import concourse.tile as tile
    from concourse import mybir
    from concourse.masks import make_identity

    f32 = mybir.dt.float32
    f32r = mybir.dt.float32r
    Alu = mybir.AluOpType
    Act = mybir.ActivationFunctionType

    nc = bass.Bass("TRN2", target_bir_lowering=False, debug=False)

    WO = 2 * GO + VO  # 432: q|k|v projection columns fused
    xT = nc.dram_tensor("xT", [D, S], f32, kind="ExternalInput").ap()
    wT = nc.dram_tensor("wT", [D, WO], f32, kind="ExternalInput").ap()
    bias = nc.dram_tensor("bias", [1, WO], f32, kind="ExternalInput").ap()
    u_d = {}
    for j in range(NH):
        for nm in ("uq1", "uq2", "uk1", "uk2"):
            u_d[(nm, j)] = nc.dram_tensor(
                f"{nm}_{j}", [S, DH], f32, kind="ExternalInput"
            ).ap()
    out_d = nc.dram_tensor("out", [S, VO], f32, kind="ExternalOutput").ap()

    with tile.TileContext(nc) as tc:
        with (
            tc.tile_pool(name="const", bufs=1) as const_pool,
            tc.tile_pool(name="persist", bufs=1) as persist,
            tc.tile_pool(name="work", bufs=2) as work,
            tc.tile_pool(name="small", bufs=4) as small,
        ):
            identity = const_pool.tile([P, P], f32)
            make_identity(nc, identity)
            onesc = const_pool.tile([P, 1], f32)
            nc.vector.memset(onesc, 1.0)
            ones1 = const_pool.tile([1, P], f32)
            nc.vector.memset(ones1, 1.0)

            w_sb = persist.tile([P, KT, WO], f32)
            nc.sync.dma_start(out=w_sb, in_=wT.rearrange("(k p) o -> p k o", p=P))
            b_sb = persist.tile([1, WO], f32)
            nc.sync.dma_start(out=b_sb, in_=bias)
            xT_sb = persist.tile([P, KT, S], f32)
            xv = xT.rearrange("(k p) s -> p k s", p=P)
            for g4 in range(4):
                ssl = slice(g4 * (S // 4), (g4 + 1) * (S // 4))
                nc.sync.dma_start(out=xT_sb[:, :, ssl], in_=xv[:, :, ssl])

            p_q = persist.tile([P, ST, GO], f32)
            p_k = persist.tile([P, ST, GO], f32)
            VW = 32  # PV stationary padded to a 32-col group
            zeroc = const_pool.tile([P, 1], f32)
            nc.vector.memset(zeroc, 0.0)
            v_all = persist.tile([P, ST, NH, VW], f32r)
            nc.vector.tensor_copy(
                v_all[:, :, :, DV + 1 : VW],
                zeroc[:, None, None, :].to_broadcast([P, ST, NH, VW - DV - 1]),
            )
            nc.vector.tensor_copy(
                v_all[:, :, :, DV : DV + 1],
                onesc[:, None, None, :].to_broadcast([P, ST, NH, 1]),
            )

            # ---------------- phase 1: fused q|k|v projection ---------------
            with tc.tile_pool(name="pjp", bufs=3, space="PSUM") as pjp:
                for st in range(ST):
                    xs = xT_sb[:, :, st * P : (st + 1) * P]
                    pp = pjp.tile([P, WO], f32, name=f"pp{st}", tag="pj")
                    for ki in range(KT):
                        nc.tensor.matmul(
                            pp,
                            lhsT=xs[:, ki, :],
                            rhs=w_sb[:, ki, :],
                            start=(ki == 0),
                            stop=False,
                        )
                    nc.tensor.matmul(pp, lhsT=ones1, rhs=b_sb, start=False, stop=True)
                    nc.scalar.activation(p_q[:, st, :], pp[:, 0:GO], Act.Tanh)
                    nc.scalar.activation(p_k[:, st, :], pp[:, GO : 2 * GO], Act.Tanh)
                    nc.scalar.activation(
                        p_q[:, st, :], p_q[:, st, :], Act.Copy, bias=0.5, scale=0.5
                    )
                    nc.scalar.activation(
                        p_k[:, st, :], p_k[:, st, :], Act.Copy, bias=0.5, scale=0.5
                    )
                    nc.vector.tensor_copy(
                        out=v_all[:, st, :, 0:DV],
                        in_=pp[:, 2 * GO : WO].rearrange("p (h v) -> p h v", h=NH),
                    )

            # ---------------- phase 2: per-head quantise + attention --------
            with (
                tc.tile_pool(name="trp", bufs=2, space="PSUM") as trp,
                tc.tile_pool(name="ssp", bufs=2, space="PSUM") as ssp,
                tc.tile_pool(name="osp", bufs=2, space="PSUM") as osp,
            ):
                for j in range(NH):
                    qskT = {}
                    for side in ("q", "k"):
                        p_all = p_q if side == "q" else p_k
                        p_h = p_all[:, :, j * DH : (j + 1) * DH]
                        u1 = work.tile([P, ST, DH], f32, name=f"u1{side}{j}", tag=f"u1{side}")
                        nc.sync.dma_start(
                            out=u1,
                            in_=u_d[(f"u{side}1", j)].rearrange("(t p) d -> p t d", p=P),
                        )
                        u2 = work.tile([P, ST, DH], f32, name=f"u2{side}{j}", tag=f"u2{side}")
                        nc.sync.dma_start(
                            out=u2,
                            in_=u_d[(f"u{side}2", j)].rearrange("(t p) d -> p t d", p=P),
                        )
                        b1 = work.tile([P, ST, DH], f32, name=f"b1{side}{j}", tag=f"b1{side}")
                        b2 = work.tile([P, ST, DH], f32, name=f"b2{side}{j}", tag=f"b2{side}")
                        if side == "q":
                            qs2 = work.tile([P, ST, 2, DH], f32, name=f"qs2{j}", tag="qs2")
                        # quantise in groups of 4 s-tiles so the first
                        # transposes/matmuls can start while later groups are
                        # still on DVE (kills the serial head-0 bubble)
                        NG, GW = 4, ST // 4
                        for g in range(NG):
                            sl = slice(g * GW, (g + 1) * GW)
                            p_g = p_h[:, sl, :]
                            b1g, b2g, u1g, u2g = b1[:, sl, :], b2[:, sl, :], u1[:, sl, :], u2[:, sl, :]
                            nc.vector.tensor_tensor(b1g, u1g, p_g, Alu.is_lt)
                            nc.vector.tensor_tensor(b2g, u2g, p_g, Alu.is_lt)
                            nc.vector.tensor_tensor(u1g, p_g, b1g, Alu.subtract)
                            d1 = small.tile([P, GW], f32, name=f"d1{side}{j}{g}", tag=f"d1{side}")
                            nc.vector.tensor_reduce(
                                d1, u1g, op=Alu.add, axis=mybir.AxisListType.X,
                                apply_absolute_value=True,
                            )
                            nc.vector.tensor_tensor(u2g, p_g, b2g, Alu.subtract)
                            d2 = small.tile([P, GW], f32, name=f"d2{side}{j}{g}", tag=f"d2{side}")
                            nc.vector.tensor_reduce(
                                d2, u2g, op=Alu.add, axis=mybir.AxisListType.X,
                                apply_absolute_value=True,
                            )
                            # d = mean + 1e-12 ; w1 = d2/(d1+d2), w2 = d1/(d1+d2)
                            nc.vector.tensor_scalar(
                                out=d1, in0=d1, scalar1=1.0 / DH, scalar2=1e-12,
                                op0=Alu.mult, op1=Alu.add,
                            )
                            nc.vector.tensor_scalar(
                                out=d2, in0=d2, scalar1=1.0 / DH, scalar2=1e-12,
                                op0=Alu.mult, op1=Alu.add,
                            )
                            ds = small.tile([P, GW], f32, name=f"ds{side}{j}{g}", tag=f"ds{side}")
                            nc.vector.tensor_tensor(ds, d1, d2, Alu.add)
                            nc.vector.reciprocal(ds, ds)
                            w1 = small.tile([P, GW], f32, name=f"w1{side}{j}{g}", tag=f"w1{side}")
                            nc.vector.tensor_tensor(w1, d2, ds, Alu.mult)
                            w2 = small.tile([P, GW], f32, name=f"w2{side}{j}{g}", tag=f"w2{side}")
                            nc.vector.tensor_tensor(w2, d1, ds, Alu.mult)
                            # qs = 2*(b1*w1 + b2*w2) - 1
                            nc.vector.tensor_tensor(
                                b1g, b1g, w1[:, :, None].to_broadcast([P, GW, DH]), Alu.mult
                            )
                            nc.vector.tensor_tensor(
                                b2g, b2g, w2[:, :, None].to_broadcast([P, GW, DH]), Alu.mult
                            )
                            if side == "q":
                                q2g = qs2[:, sl, :, :]
                                nc.vector.tensor_tensor(q2g[:, :, 0, :], b1g, b2g, Alu.add)
                                nc.vector.tensor_scalar(
                                    out=q2g[:, :, 0, :], in0=q2g[:, :, 0, :],
                                    scalar1=2.0, scalar2=-1.0,
                                    op0=Alu.mult, op1=Alu.add,
                                )
                                nc.vector.tensor_copy(q2g[:, :, 1, :], q2g[:, :, 0, :])
                            else:
                                nc.vector.tensor_tensor(b1g, b1g, b2g, Alu.add)
                                nc.vector.tensor_scalar(
                                    out=b1g, in0=b1g, scalar1=2.0, scalar2=-1.0,
                                    op0=Alu.mult, op1=Alu.add,
                                )
                        qs = b1
                        # Transpose [s, dh] -> [dh, s] via PE on [128,128]
                        # blocks.  For q: transpose a free-dim-duplicated tile
                        # so qsT lands identically on partitions 0-63 and
                        # 64-127 (feeds both concurrent row-group matmuls).
                        # For k: transpose s-tile PAIRS so even k-tiles land on
                        # partitions 0-63 and odd ones on 64-127 (packed ksT).
                        if side == "q":
                            qsT = work.tile([P, S], f32r, name=f"qsT{j}", tag="qsT")
                            for st in range(ST):
                                tr = trp.tile([P, P], f32, name=f"trq{j}{st}", tag="tr")
                                nc.tensor.transpose(tr, qs2[:, st, :, :], identity)
                                nc.vector.tensor_copy(
                                    out=qsT[:, st * P : (st + 1) * P], in_=tr
                                )
                            qskT[side] = qsT
                        else:
                            ksT = work.tile([P, ST // 2, P], f32r, name=f"ksT{j}", tag="ksT")
                            for kp in range(ST // 2):
                                tr = trp.tile([P, P], f32, name=f"trk{j}{kp}", tag="tr")
                                nc.tensor.transpose(
                                    tr, qs[:, 2 * kp : 2 * kp + 2, :], identity
                                )
                                nc.vector.tensor_copy(out=ksT[:, kp, :], in_=tr)
                            qskT[side] = ksT

                    qsT, ksT = qskT["q"], qskT["k"]
                    # attention for head j: scores row-packed (two k-tiles in
                    # row groups 0/64 concurrently), PV accumulates [32, QW]
                    for qc in range(QC):
                        rhs_q = qsT[:, qc * QW : (qc + 1) * QW]
                        o_ps = osp.tile([32, QW], f32, name=f"o{j}{qc}", tag="o")
                        for kp in range(ST // 2):
                            s_ps = ssp.tile([P, 2, QW], f32, name=f"s{j}{qc}{kp}", tag="s")
                            for h2 in range(2):
                                base = h2 * DH
                                nc.tensor.matmul(
                                    s_ps[:, h2, :],
                                    lhsT=ksT[base : base + DH, kp, :],
                                    rhs=rhs_q[base : base + DH, :],
                                    start=True,
                                    stop=True,
                                )
                            p_sb = work.tile([P, 2, QW], f32r, name=f"p{j}{qc}{kp}", tag="p")
                            nc.scalar.activation(p_sb, s_ps, Act.Exp, scale=SCALE)
                            for h2 in range(2):
                                kt = kp * 2 + h2
                                nc.tensor.matmul(
                                    o_ps,
                                    lhsT=v_all[:, kt, j, :],
                                    rhs=p_sb[:, h2, :],
                                    start=(kt == 0),
                                    stop=(kt == ST - 1),
                                )
                        oT = work.tile([DV + 1, QW], f32, name=f"oT{j}{qc}", tag="oT")
                        nc.vector.tensor_copy(oT, o_ps[0 : DV + 1, :])
                        for blk in range(QW // P):
                            tro = trp.tile([P, DV + 1], f32, name=f"tro{j}{qc}{blk}", tag="tr")
                            nc.tensor.transpose(
                                tro,
                                oT[:, blk * P : (blk + 1) * P],
                                identity[: DV + 1, : DV + 1],
                            )
                            rec = small.tile([P, 1], f32, name=f"rec{j}{qc}{blk}", tag="rec")
                            nc.vector.reciprocal(rec, tro[:, DV : DV + 1])
                            o_sb = small.tile([P, DV], f32, name=f"ob{j}{qc}{blk}", tag="ob")
                            nc.vector.tensor_scalar_mul(o_sb, tro[:, 0:DV], rec)
                            s0 = qc * QW + blk * P
                            nc.sync.dma_start(
                                out=out_d[s0 : s0 + P, j * DV : (j + 1) * DV],
                                in_=o_sb,
                            )
    _split_multiwaits(nc)
    return nc


_NC = None


def _get_nc():
    global _NC
    if _NC is None:
        _patch_tile_tail_drain()
        _NC = _build_nc()
    return _NC


def _shard_inputs(inputs):
    x = np.asarray(inputs["x"], dtype=np.float32)
    Wq = np.asarray(inputs["Wq"], dtype=np.float32)
    bq = np.asarray(inputs["bq"], dtype=np.float32)
    Wk = np.asarray(inputs["Wk"], dtype=np.float32)
    bk = np.asarray(inputs["bk"], dtype=np.float32)
    Wv = np.asarray(inputs["Wv"], dtype=np.float32)
    bv = np.asarray(inputs["bv"], dtype=np.float32)
    us = {nm: np.asarray(inputs[nm], dtype=np.float32)
          for nm in ("u_q1", "u_q2", "u_k1", "u_k2")}

    in_maps = []
    for c in range(NCORES):
        b, g = divmod(c, 2)
        wT = np.concatenate(
            [
                Wq[g * GO : (g + 1) * GO, :].T,
                Wk[g * GO : (g + 1) * GO, :].T,
                Wv[g * VO : (g + 1) * VO, :].T,
            ],
            axis=1,
        )
        bias = np.concatenate(
            [
                bq[g * GO : (g + 1) * GO],
                bk[g * GO : (g + 1) * GO],
                bv[g * VO : (g + 1) * VO],
            ]
        ).reshape(1, -1)
        m = {
            "xT": np.ascontiguousarray(x[b].T),
            "wT": np.ascontiguousarray(wT),
            "bias": np.ascontiguousarray(bias),
        }
        for j in range(NH):
            bh = b * H + g * NH + j
            m[f"uq1_{j}"] = np.ascontiguousarray(us["u_q1"][bh])
            m[f"uq2_{j}"] = np.ascontiguousarray(us["u_q2"][bh])
            m[f"uk1_{j}"] = np.ascontiguousarray(us["u_k1"][bh])
            m[f"uk2_{j}"] = np.ascontiguousarray(us["u_k2"][bh])
        in_maps.append(m)
    return in_maps


def _run(inputs, trace=False, tmpdir=None):
    from concourse.bass_utils import run_bass_kernel_spmd

    if trace:
        _install_ntff_hook()
    nc = _get_nc()
    in_maps = _shard_inputs(inputs)
    kw = {}
    if trace:
        kw["trace"] = True
        if tmpdir is not None:
            kw["tmpdir"] = tmpdir
    res = run_bass_kernel_spmd(nc, in_maps, core_ids=list(range(NCORES)), **kw)
    out = np.zeros((B, S, H * DV), dtype=np.float32)
    for c in range(NCORES):
        b, g = divmod(c, 2)
        out[b, :, g * VO : (g + 1) * VO] = res.results[c]["out"]
    return (out,), res


def kernel(**inputs):
    out, _ = _run(inputs, trace=False)
    return out


def kernel_profiled(tmpdir=None, **inputs):
    out, res = _run(inputs, trace=True, tmpdir=tmpdir)
    return out, res.exec_time_ns
